# revision 49
# baseline (speedup 1.0000x reference)
"""Trainium2 Bass kernel for nn_PoolHiddenNet (gnn_message_passing).

Reference computation (uniform contiguous groups of P=16):
    pos = in_xy[-1]                       # (B, 2)
    rel[g,i,j] = pos[g,j] - pos[g,i]
    emb = rel @ W_emb + b_emb             # (G,P,P,E)
    x   = concat([emb, h[g,j]], -1)
    x1  = relu(x @ W1 + b1)               # (G,P,P,H)
    x2  = relu(x1 @ W2 + b2)              # (G,P,P,BOT)
    out = max over j -> (B, BOT)

Algebraic restructuring used here:
    x1[g,i,j] = relu(u[g,j] - v[g,i])
       u[g,r]  = pos[g,r] @ (W_emb @ W1[:E]) + h[g,r] @ W1[E:] + (b_emb @ W1[:E] + b1)
       v[g,r]  = pos[g,r] @ (W_emb @ W1[:E])
    out[g,i]  = max_j relu(x1[g,i,j] @ W2 + b2)      (relu commutes with max)

Sharding: data-parallel over groups; 64 groups (1024 rows) per core.
Device layout: "dup-halves" -- SBUF partitions 0:64 carry the h-dim for the
first 32 groups' data, partitions 64:128 carry the h-dim for the last 32
groups, so all DVE/ACT ops use the full 128 lanes.

The W2 matmuls contract K=128 against zero-padded weights (W2z cols
0:BOT = [W2;0] selects the half-0 x1 rows, cols BOT:2BOT = [0;W2] the
half-1 rows).  Measured on hw, K<=64 matmuls run the PE at half column
rate (427ns vs 216ns per 512 output columns), so padding the contraction
to 128 rows doubles PE throughput for free and drops the tensor engine
from ~103us busy (the previous 3-way bottleneck) to ~75us, leaving ACT
and DVE as the only walls.

Drain schedule (the true bottleneck): every z element must cross
PSUM->SBUF through ACT (1 elem/cycle @1.2GHz) or DVE (1 elem/cycle
@0.96GHz); nothing else on TRN2 can read PSUM (dual-PSUM tensor_tensor
operands are rejected by the bir verifier, DMA cannot read PSUM, and
GpSimd tensor ops fail codegen in this toolchain).  Each PSUM tile
[128, 1024] fp32 is one (bot-chunk c, k-half, hh) = (64 gi x 16 j); a
4-deep PSUM ring keeps the PE ahead of the drains.  Routes:
  ACT (c in {0,1,2,4,5,6}): relu+bias ACTIVATE drains the tile to SBUF
      bf16 y blocks; the 16->1 max tree runs on DVE as bf16 2x
      tensor_tensor ops batched 6 blocks at a time, L4 writing straight
      into pooled.
  DVE (c in {3,7}): reduce_max straight from PSUM (raw), fused
      (add bias, max 0) tensor_scalar afterwards.
The emission order interleaves DVE-routed tiles between ACT c's and
places each tree burst where the ring never waits on a queued DVE
reduce; the previous sbp's last tree fills the DVE idle at each sbp
boundary.  GpSimd does nothing; all DMA runs on sync/scalar queues.
Both ACT and DVE end up ~95% busy; this schedule sits at the measured
ACT+DVE drain wall (~26-27us per superblock-pair).
"""

import sys

import numpy as np

try:
    import concourse.bass as bass
except ImportError:  # pragma: no cover
    sys.path.insert(0, "/opt/trn_rl_repo")
    import concourse.bass as bass

from concourse import bacc

import ml_dtypes

import concourse.mybir as mybir
from concourse.bass_utils import run_bass_kernel_spmd
from concourse.tile import TileContext

# Problem constants (hardcoded per spec)
B, G, P, E, H, BOT = 8192, 512, 16, 64, 64, 1024
NCORES = 8
GC = G // NCORES  # 64 groups per core
RC = GC * P  # 1024 batch rows per core
HALF_ROWS = RC // 2  # 512 rows per half
HALF_PAIRS = (GC // 2) * P * P  # 8192 pairs per half
NSBP = 4  # superblock-pairs; each sbp makes one 128-row block per half
SB_PAIRS = 2048  # pairs per (sbp, half) = 128 rows * 16 j
BCH = BOT // 128  # 8 bot chunks of 128 channels

FP = mybir.dt.float32
BF = mybir.dt.bfloat16

# bot-chunks drained by DVE reduce_max; the rest go to ACT.
DVE_CS = (3, 7)
# ACT c's grouped into tree batches; each batch is a contiguous run of
# pooled columns (pooled col = c*128 + k*64).  The last sbp splits the
# second batch so the post-act6 DVE tail is one small burst instead of
# the full (4,5,6) ladder.
TREE_BATCHES = ((0, 1, 2), (4, 5, 6))
TREE_BATCHES_LAST = ((0, 1, 2), (4, 5), (6,))
# fixed y-block index per ACT c (independent of tree batching)
BLOCK_BI = {0: 0, 1: 1, 2: 2, 4: 3, 5: 4, 6: 5}
NB = 12  # y blocks of 1024 per (sbp, hh)

_CACHE = {}


def build_nc():
    nc = bacc.Bacc("TRN2", target_bir_lowering=False, debug=False, num_devices=NCORES)
    posT_d = nc.declare_dram_parameter("posT", [2, RC], BF, isOutput=False)
    hT_d = nc.declare_dram_parameter("hT", [H, RC], BF, isOutput=False)
    A_d = nc.declare_dram_parameter("Amat", [2, H], BF, isOutput=False)
    W1b_d = nc.declare_dram_parameter("W1b", [H, H], BF, isOutput=False)
    c0_d = nc.declare_dram_parameter("c0d", [128, 1], FP, isOutput=False)
    W2_d = nc.declare_dram_parameter("W2d", [128, 2 * BOT], BF, isOutput=False)
    b2_d = nc.declare_dram_parameter("b2s", [128, BCH], FP, isOutput=False)
    out_d = nc.declare_dram_parameter("out", [BOT, RC], BF, isOutput=True)

    with TileContext(nc) as tc:
        with (
            tc.tile_pool(name="const", bufs=1) as constp,
            tc.tile_pool(name="big", bufs=1) as bigp,
            tc.tile_pool(name="y", bufs=4) as yp,
            tc.tile_pool(name="tree", bufs=2) as treep,
            tc.tile_pool(name="outp", bufs=3) as outp,
        ):
            # ---- constants / inputs to SBUF (split across two DMA queues;
            # gpsimd stays idle so its DGE drain is cheap) ----
            posT = constp.tile([2, RC], BF)
            nc.scalar.dma_start(posT, posT_d[:, :])
            hT = constp.tile([H, RC], BF)
            nc.scalar.dma_start(hT, hT_d[:, :])
            A_sb = constp.tile([2, H], BF)
            nc.sync.dma_start(A_sb, A_d[:, :])
            W1b_sb = constp.tile([H, H], BF)
            nc.sync.dma_start(W1b_sb, W1b_d[:, :])
            c0_sb = constp.tile([128, 1], FP)
            nc.sync.dma_start(c0_sb, c0_d[:, :])
            b2_sb = constp.tile([128, BCH], FP)
            nc.sync.dma_start(b2_sb, b2_d[:, :])
            W2_sb = constp.tile([128, 2 * BOT], BF)
            nc.sync.dma_start(W2_sb, W2_d[:, :])

            # ---- u/v prep (dup-halves layout) ----
            # uT[p, r'] : h = p % 64 ; r = (p // 64) * 512 + r'
            with tc.tile_pool(name="prepps", bufs=1, space="PSUM") as prepps:
                psum_u = prepps.tile([128, HALF_ROWS], FP)
                vT = constp.tile([128, HALF_ROWS], FP)
                uT = constp.tile([128, HALF_ROWS], FP)
                # v = pos @ A for both halves, one copy out (ACT), then the
                # h-term accumulates on top; uadd runs on DVE.  Done in two
                # row-slices so sbp0's rows (0:128) finish first and the x1
                # build can start early.
                for r0, r1 in ((0, 128), (128, HALF_ROWS)):
                    for hh in range(2):
                        usl = psum_u[64 * hh : 64 * (hh + 1), r0:r1]
                        rs = slice(hh * HALF_ROWS + r0, hh * HALF_ROWS + r1)
                        nc.tensor.matmul(
                            usl, A_sb, posT[:, rs],
                            start=True, stop=True, tile_position=(0, 64 * hh),
                        )
                    nc.scalar.copy(vT[:, r0:r1], psum_u[:, r0:r1])
                    for hh in range(2):
                        usl = psum_u[64 * hh : 64 * (hh + 1), r0:r1]
                        rs = slice(hh * HALF_ROWS + r0, hh * HALF_ROWS + r1)
                        nc.tensor.matmul(
                            usl, W1b_sb, hT[:, rs],
                            start=False, stop=True, tile_position=(0, 64 * hh),
                            skip_group_check=True,
                        )
                    # uT = psum_u + c0 (per-partition bias)
                    nc.vector.tensor_scalar_add(
                        uT[:, r0:r1], psum_u[:, r0:r1], c0_sb
                    )

            # ---- X1T = relu(u[g,j] - v[g,i]) as bf16, pairs = (g, i, j) ----
            x1 = bigp.tile([128, HALF_PAIRS], BF)
            x1p = bigp.tile([128, HALF_PAIRS], BF)
            GSB = GC // 2 // NSBP  # groups per (sbp, half) = 8
            chunks = [(0, 2), (2, GSB // 2), (GSB // 2, GSB)] + [
                (sbp * GSB, (sbp + 1) * GSB) for sbp in range(1, NSBP)
            ]

            def emit_x1_chunk(g0, g1):
                ng = g1 - g0
                gs = slice(g0 * P, g1 * P)
                ps = slice(g0 * P * P, g1 * P * P)
                u3 = uT[:, gs].rearrange("p (g t) -> p g t", t=P)  # t = j
                u4 = u3.unsqueeze(2).broadcast_to([128, ng, P, P])
                v3 = vT[:, gs].rearrange("p (g t) -> p g t", t=P)  # t = i
                v4 = v3.unsqueeze(3).broadcast_to([128, ng, P, P])
                x1p4 = x1p[:, ps].rearrange("p (g i j) -> p g i j", i=P, j=P)
                nc.vector.tensor_tensor(x1p4, u4, v4, op=mybir.AluOpType.subtract)
                nc.vector.tensor_scalar_max(x1[:, ps], x1p[:, ps], 0.0)

            # first three chunks (= sbp0's pairs) up front; later chunks are
            # interleaved into the main loop so DVE work stays spread out
            for g0, g1 in chunks[:3]:
                emit_x1_chunk(g0, g1)

            # ---- main loop ----
            # output is written TRANSPOSED (out_d[bot, row], bf16); the host
            # untransposes during unshard.
            outT = out_d.rearrange("(c p) r -> p c r", p=128)
            with tc.tile_pool(name="psz", bufs=4, space="PSUM") as psz:
                y_hist = {}  # sbp -> [y_t per hh]
                pool_hist = {}  # sbp -> pooledT tile ([128, 2048], hh-split)

                def emit_mms(sbp, c, k):
                    """Fill the two hh tiles of (c, k); K=128 matmuls against
                    the zero-padded weight halves."""
                    ts = [
                        psz.tile([128, 1024], FP, tag="z", name="zt")
                        for _ in range(2)
                    ]
                    for n in range(2):
                        pbase = sbp * SB_PAIRS + k * 1024 + n * 512
                        for hh in range(2):
                            wbase = hh * BOT + c * 128
                            nc.tensor.matmul(
                                ts[hh][:, n * 512 : (n + 1) * 512],
                                W2_sb[:, wbase : wbase + 128],
                                x1[:, pbase : pbase + 512],
                                start=True,
                                stop=True,
                            )
                    return ts

                def emit_act(sbp, c):
                    y_t = y_hist[sbp]
                    bi = BLOCK_BI[c]
                    for k in range(2):
                        ts = emit_mms(sbp, c, k)
                        for hh in range(2):
                            b = bi * 2 + k
                            nc.scalar.activation(
                                y_t[hh][:, b * 1024 : (b + 1) * 1024],
                                ts[hh],
                                mybir.ActivationFunctionType.Relu,
                                bias=b2_sb[:, c : c + 1],
                                scale=1.0,
                            )

                def emit_dve(sbp, c, k):
                    pooledT = pool_hist[sbp]
                    ts = emit_mms(sbp, c, k)
                    for hh in range(2):
                        psl = pooledT[
                            :,
                            hh * 1024 + c * 128 + k * 64 :
                            hh * 1024 + c * 128 + (k + 1) * 64,
                        ]
                        nc.vector.reduce_max(
                            psl,
                            ts[hh].rearrange("p (u j) -> p u j", j=P),
                            axis=mybir.AxisListType.X,
                        )

                def emit_bias(sbp, c, on_act=False):
                    # relu(x + b2) in place; on_act routes it to the scalar
                    # engine (used where ACT has a known sem-wait bubble)
                    pooledT = pool_hist[sbp]
                    for hh in range(2):
                        csl = pooledT[
                            :, hh * 1024 + c * 128 : hh * 1024 + (c + 1) * 128
                        ]
                        if on_act:
                            nc.scalar.activation(
                                csl, csl,
                                mybir.ActivationFunctionType.Relu,
                                bias=b2_sb[:, c : c + 1],
                                scale=1.0,
                            )
                        else:
                            nc.vector.tensor_scalar(
                                csl, csl, b2_sb[:, c : c + 1], 0.0,
                                op0=mybir.AluOpType.add,
                                op1=mybir.AluOpType.max,
                            )

                def emit_tree(sbp, bt, hh):
                    batches = (
                        TREE_BATCHES_LAST if sbp == NSBP - 1 else TREE_BATCHES
                    )
                    cs = batches[bt]
                    y_t = y_hist[sbp]
                    pooledT = pool_hist[sbp]
                    nb = len(cs) * 2
                    m = nb * 64
                    yb0 = BLOCK_BI[cs[0]] * 2 * 1024
                    Y = y_t[hh][:, yb0 : yb0 + nb * 1024].rearrange(
                        "p (m j) -> p m j", j=16
                    )
                    t1 = treep.tile([128, m * 8], BF, tag="t1", name="t1")
                    T1 = t1.rearrange("p (m j) -> p m j", j=8)
                    nc.vector.tensor_tensor(
                        T1, Y[:, :, 0:8], Y[:, :, 8:16], op=mybir.AluOpType.max
                    )
                    t2 = treep.tile([128, m * 4], BF, tag="t2", name="t2")
                    T2 = t2.rearrange("p (m j) -> p m j", j=4)
                    nc.vector.tensor_tensor(
                        T2, T1[:, :, 0:4], T1[:, :, 4:8], op=mybir.AluOpType.max
                    )
                    t3 = treep.tile([128, m * 2], BF, tag="t3", name="t3")
                    T3 = t3.rearrange("p (m j) -> p m j", j=2)
                    nc.vector.tensor_tensor(
                        T3, T2[:, :, 0:2], T2[:, :, 2:4], op=mybir.AluOpType.max
                    )
                    p0 = hh * 1024 + cs[0] * 128
                    nc.vector.tensor_tensor(
                        pooledT[:, p0 : p0 + m], T3[:, :, 0], T3[:, :, 1],
                        op=mybir.AluOpType.max,
                    )

                def emit_out_dma(sbp, hh, c0, c1):
                    """Ship bot-chunks [c0, c1) of half hh."""
                    pooledT = pool_hist[sbp]
                    rowbase = hh * HALF_ROWS + sbp * 128
                    nc.sync.dma_start(
                        outT[:, c0:c1, rowbase : rowbase + 128],
                        pooledT[
                            :, hh * 1024 + c0 * 128 : hh * 1024 + c1 * 128
                        ].rearrange("p (c u) -> p c u", c=c1 - c0),
                    )

                for sbp in range(NSBP):
                    y_hist[sbp] = [
                        yp.tile([128, NB * 1024], BF, tag="y", name="y_t")
                        for _ in range(2)
                    ]
                    pool_hist[sbp] = outp.tile(
                        [128, 2048], BF, tag="pooledT", name="pooledT"
                    )
                    # Emission order doubles as the PSUM ring order and the
                    # per-engine queue order; DVE reduces are placed so they
                    # never sit behind more than ~2us of queued tree work,
                    # keeping the 4-deep ring from stalling ACT.
                    emit_act(sbp, 0)
                    emit_dve(sbp, 3, 0)
                    emit_act(sbp, 1)
                    if sbp > 0:
                        # prev sbp's last tree slots between the c3 reduces
                        emit_tree(sbp - 1, 1, 1)
                        emit_out_dma(sbp - 1, 1, 4, 7)
                    emit_dve(sbp, 3, 1)
                    emit_act(sbp, 2)
                    emit_tree(sbp, 0, 0)
                    if sbp + 3 < len(chunks):
                        # fills DVE idle between T0h0 and the D7 reduces
                        g0, g1 = chunks[sbp + 3]
                        gm = (g0 + g1) // 2
                        emit_x1_chunk(g0, gm)
                        emit_x1_chunk(gm, g1)
                    emit_act(sbp, 4)
                    emit_act(sbp, 5)
                    emit_dve(sbp, 7, 0)
                    emit_dve(sbp, 7, 1)
                    emit_tree(sbp, 0, 1)
                    emit_bias(sbp, 3, on_act=True)
                    emit_bias(sbp, 7)
                    emit_out_dma(sbp, 0, 7, 8)
                    emit_out_dma(sbp, 1, 7, 8)
                    # first halves (c0-c3) of both hh are final now; ship them
                    emit_out_dma(sbp, 0, 0, 4)
                    emit_out_dma(sbp, 1, 0, 4)
                    if sbp < NSBP - 1:
                        emit_act(sbp, 6)
                        emit_tree(sbp, 1, 0)
                        emit_out_dma(sbp, 0, 4, 7)
                    else:
                        # last sbp: the (4,5) trees run before act6's
                        # drains land, leaving only the small (6,) ladders
                        # (+DMA) as the exposed DVE tail.
                        emit_tree(sbp, 1, 0)
                        emit_act(sbp, 6)
                        emit_tree(sbp, 1, 1)
                        emit_tree(sbp, 2, 0)
                        emit_out_dma(sbp, 0, 4, 7)
                        emit_tree(sbp, 2, 1)
                        emit_out_dma(sbp, 1, 4, 7)
    nc.finalize()
    return nc


def _get_nc():
    if "nc" not in _CACHE:
        _CACHE["nc"] = build_nc()
    return _CACHE["nc"]


def kernel(
    in_xy, in_dxdy, h_states, seq_start_end, W_emb, b_emb, W1, b1, W2, b2
):
    pos = np.asarray(in_xy, dtype=np.float32)[-1]  # (B, 2)
    hs = np.asarray(h_states, dtype=np.float32).reshape(B, H)
    W_emb = np.asarray(W_emb, dtype=np.float32)
    b_emb = np.asarray(b_emb, dtype=np.float32)
    W1 = np.asarray(W1, dtype=np.float32)
    b1 = np.asarray(b1, dtype=np.float32)
    W2 = np.asarray(W2, dtype=np.float32)
    b2 = np.asarray(b2, dtype=np.float32)

    A = np.ascontiguousarray(W_emb @ W1[:E])  # (2, H)
    W1b = np.ascontiguousarray(W1[E:])  # (H, H)
    c0 = b_emb @ W1[:E] + b1  # (H,)
    c0d = np.ascontiguousarray(np.concatenate([c0, c0])[:, None])  # (128,1)
    W2z = np.zeros((128, 2 * BOT), np.float32)
    W2z[0:64, 0:BOT] = W2
    W2z[64:128, BOT : 2 * BOT] = W2
    W2d = np.ascontiguousarray(W2z.astype(ml_dtypes.bfloat16))  # (128, 2*BOT)
    b2s = np.ascontiguousarray(b2.reshape(BCH, 128).T)  # (128, BCH)

    in_maps = []
    for cid in range(NCORES):
        rs = slice(cid * RC, (cid + 1) * RC)
        in_maps.append(
            {
                "posT": np.ascontiguousarray(pos[rs].T).astype(ml_dtypes.bfloat16),
                "hT": np.ascontiguousarray(hs[rs].T).astype(ml_dtypes.bfloat16),
                "Amat": A.astype(ml_dtypes.bfloat16),
                "W1b": W1b.astype(ml_dtypes.bfloat16),
                "c0d": c0d,
                "W2d": W2d,
                "b2s": b2s,
            }
        )

    _CACHE["in_maps"] = in_maps
    nc = _get_nc()
    res = run_bass_kernel_spmd(nc, in_maps, core_ids=list(range(NCORES)))
    return np.concatenate(
        [np.asarray(r["out"], dtype=np.float32).T for r in res.results], axis=0
    )


if __name__ == "__main__":
    rng = np.random.default_rng(0)
    inputs = {
        "in_xy": rng.standard_normal((8, B, 2), dtype=np.float32),
        "in_dxdy": rng.standard_normal((8, B, 2), dtype=np.float32),
        "h_states": rng.standard_normal((1, B, H), dtype=np.float32),
        "seq_start_end": np.stack(
            [np.arange(G) * P, np.arange(G) * P + P], axis=1
        ).astype(np.int64),
        "W_emb": rng.standard_normal((2, E), dtype=np.float32),
        "b_emb": np.zeros(E, dtype=np.float32),
        "W1": rng.standard_normal((E + H, H), dtype=np.float32),
        "b1": np.zeros(H, dtype=np.float32),
        "W2": rng.standard_normal((H, BOT), dtype=np.float32),
        "b2": np.zeros(BOT, dtype=np.float32),
    }
    out = kernel(**inputs)
    print(out.shape, out.dtype)


# revision 50
# speedup vs baseline: 1.0173x; 1.0173x over previous
"""Trainium2 Bass kernel for nn_PoolHiddenNet (gnn_message_passing).

Reference computation (uniform contiguous groups of P=16):
    pos = in_xy[-1]                       # (B, 2)
    rel[g,i,j] = pos[g,j] - pos[g,i]
    emb = rel @ W_emb + b_emb             # (G,P,P,E)
    x   = concat([emb, h[g,j]], -1)
    x1  = relu(x @ W1 + b1)               # (G,P,P,H)
    x2  = relu(x1 @ W2 + b2)              # (G,P,P,BOT)
    out = max over j -> (B, BOT)

Algebraic restructuring used here:
    x1[g,i,j] = relu(u[g,j] - v[g,i])
       u[g,r]  = pos[g,r] @ (W_emb @ W1[:E]) + h[g,r] @ W1[E:] + (b_emb @ W1[:E] + b1)
       v[g,r]  = pos[g,r] @ (W_emb @ W1[:E])
    out[g,i]  = max_j relu(x1[g,i,j] @ W2 + b2)      (relu commutes with max)

Sharding: data-parallel over groups; 64 groups (1024 rows) per core.
Device layout: "dup-halves" -- SBUF partitions 0:64 carry the h-dim for the
first 32 groups' data, partitions 64:128 carry the h-dim for the last 32
groups, so all DVE/ACT ops use the full 128 lanes.

The W2 matmuls contract K=128 against zero-padded weights (W2z cols
0:BOT = [W2;0] selects the half-0 x1 rows, cols BOT:2BOT = [0;W2] the
half-1 rows).  Measured on hw, K<=64 matmuls run the PE at half column
rate (427ns vs 216ns per 512 output columns), so padding the contraction
to 128 rows doubles PE throughput for free and drops the tensor engine
from ~103us busy (the previous 3-way bottleneck) to ~75us, leaving ACT
and DVE as the only walls.

Drain schedule (the true bottleneck): every z element must cross
PSUM->SBUF through ACT (1 elem/cycle @1.2GHz) or DVE (1 elem/cycle
@0.96GHz); nothing else on TRN2 can read PSUM (dual-PSUM tensor_tensor
operands are rejected by the bir verifier, DMA cannot read PSUM, and
GpSimd tensor ops fail codegen in this toolchain).  Each PSUM tile
[128, 1024] fp32 is one (bot-chunk c, k-half, hh) = (64 gi x 16 j); a
4-deep PSUM ring keeps the PE ahead of the drains.  Routes:
  ACT (c in {0,1,2,4,5,6}): relu+bias ACTIVATE drains the tile to SBUF
      bf16 y blocks; the 16->1 max tree runs on DVE as bf16 2x
      tensor_tensor ops batched 6 blocks at a time, L4 writing straight
      into pooled.
  DVE (c in {3,7}): reduce_max straight from PSUM (raw), fused
      (add bias, max 0) tensor_scalar afterwards.
The emission order interleaves DVE-routed tiles between ACT c's and
places each tree burst where the ring never waits on a queued DVE
reduce; the previous sbp's last tree fills the DVE idle at each sbp
boundary.  GpSimd does nothing; all DMA runs on sync/scalar queues.
Both ACT and DVE end up ~95% busy; this schedule sits at the measured
ACT+DVE drain wall (~26-27us per superblock-pair).
"""

import sys

import numpy as np

try:
    import concourse.bass as bass
except ImportError:  # pragma: no cover
    sys.path.insert(0, "/opt/trn_rl_repo")
    import concourse.bass as bass

from concourse import bacc

import ml_dtypes

import concourse.mybir as mybir
from concourse.bass_utils import run_bass_kernel_spmd
from concourse.tile import TileContext

# Problem constants (hardcoded per spec)
B, G, P, E, H, BOT = 8192, 512, 16, 64, 64, 1024
NCORES = 8
GC = G // NCORES  # 64 groups per core
RC = GC * P  # 1024 batch rows per core
HALF_ROWS = RC // 2  # 512 rows per half
HALF_PAIRS = (GC // 2) * P * P  # 8192 pairs per half
NSBP = 4  # superblock-pairs; each sbp makes one 128-row block per half
SB_PAIRS = 2048  # pairs per (sbp, half) = 128 rows * 16 j
BCH = BOT // 128  # 8 bot chunks of 128 channels

FP = mybir.dt.float32
BF = mybir.dt.bfloat16

# bot-chunks drained by DVE reduce_max; the rest go to ACT.
DVE_CS = (3, 7)
# ACT c's grouped into tree batches; each batch is a contiguous run of
# pooled columns (pooled col = c*128 + k*64).  The last sbp splits the
# second batch so the post-act6 DVE tail is one small burst instead of
# the full (4,5,6) ladder.
TREE_BATCHES = ((0, 1, 2), (4, 5, 6))
TREE_BATCHES_LAST = ((0, 1, 2), (4, 5), (6,))
# fixed y-block index per ACT c (independent of tree batching)
BLOCK_BI = {0: 0, 1: 1, 2: 2, 4: 3, 5: 4, 6: 5}
NB = 12  # y blocks of 1024 per (sbp, hh)

_CACHE = {}


def build_nc():
    nc = bacc.Bacc("TRN2", target_bir_lowering=False, debug=False, num_devices=NCORES)
    posT_d = nc.declare_dram_parameter("posT", [2, RC], BF, isOutput=False)
    hT_d = nc.declare_dram_parameter("hT", [H, RC], BF, isOutput=False)
    A_d = nc.declare_dram_parameter("Amat", [2, H], BF, isOutput=False)
    W1b_d = nc.declare_dram_parameter("W1b", [H, H], BF, isOutput=False)
    c0_d = nc.declare_dram_parameter("c0d", [128, 1], FP, isOutput=False)
    W2_d = nc.declare_dram_parameter("W2d", [128, 2 * BOT], BF, isOutput=False)
    b2_d = nc.declare_dram_parameter("b2s", [128, BCH], FP, isOutput=False)
    out_d = nc.declare_dram_parameter("out", [BOT, RC], BF, isOutput=True)

    with TileContext(nc) as tc:
        with (
            tc.tile_pool(name="const", bufs=1) as constp,
            tc.tile_pool(name="big", bufs=1) as bigp,
            tc.tile_pool(name="y", bufs=4) as yp,
            tc.tile_pool(name="tree", bufs=2) as treep,
            tc.tile_pool(name="outp", bufs=3) as outp,
        ):
            # ---- constants / inputs to SBUF (split across two DMA queues;
            # gpsimd stays idle so its DGE drain is cheap) ----
            posT = constp.tile([2, RC], BF)
            nc.scalar.dma_start(posT, posT_d[:, :])
            # hT in two pieces: the 32KB strided slab covering prep's first
            # row-slice (cols 0:128 and 512:640 = both halves' rows 0:128)
            # lands ~2.5us before the rest of the 128KB, unblocking the
            # W1b matmul that gates the whole x1/drain chain.
            hT = constp.tile([H, RC], BF)
            hT3 = hT.rearrange("h (b c) -> h b c", c=HALF_ROWS)
            hTd3 = hT_d.rearrange("h (b c) -> h b c", c=HALF_ROWS)
            nc.scalar.dma_start(hT3[:, :, 0:128], hTd3[:, :, 0:128])
            nc.scalar.dma_start(hT3[:, :, 128:HALF_ROWS], hTd3[:, :, 128:HALF_ROWS])
            A_sb = constp.tile([2, H], BF)
            nc.sync.dma_start(A_sb, A_d[:, :])
            W1b_sb = constp.tile([H, H], BF)
            nc.sync.dma_start(W1b_sb, W1b_d[:, :])
            c0_sb = constp.tile([128, 1], FP)
            nc.sync.dma_start(c0_sb, c0_d[:, :])
            b2_sb = constp.tile([128, BCH], FP)
            nc.sync.dma_start(b2_sb, b2_d[:, :])
            W2_sb = constp.tile([128, 2 * BOT], BF)
            nc.sync.dma_start(W2_sb, W2_d[:, :])

            # ---- u/v prep (dup-halves layout) ----
            # uT[p, r'] : h = p % 64 ; r = (p // 64) * 512 + r'
            with tc.tile_pool(name="prepps", bufs=1, space="PSUM") as prepps:
                psum_u = prepps.tile([128, HALF_ROWS], FP)
                vT = constp.tile([128, HALF_ROWS], FP)
                uT = constp.tile([128, HALF_ROWS], FP)
                # v = pos @ A for both halves, one copy out (ACT), then the
                # h-term accumulates on top; uadd runs on DVE.  Done in two
                # row-slices so sbp0's rows (0:128) finish first and the x1
                # build can start early.
                for r0, r1 in ((0, 128), (128, HALF_ROWS)):
                    for hh in range(2):
                        usl = psum_u[64 * hh : 64 * (hh + 1), r0:r1]
                        rs = slice(hh * HALF_ROWS + r0, hh * HALF_ROWS + r1)
                        nc.tensor.matmul(
                            usl, A_sb, posT[:, rs],
                            start=True, stop=True, tile_position=(0, 64 * hh),
                        )
                    nc.scalar.copy(vT[:, r0:r1], psum_u[:, r0:r1])
                    for hh in range(2):
                        usl = psum_u[64 * hh : 64 * (hh + 1), r0:r1]
                        rs = slice(hh * HALF_ROWS + r0, hh * HALF_ROWS + r1)
                        nc.tensor.matmul(
                            usl, W1b_sb, hT[:, rs],
                            start=False, stop=True, tile_position=(0, 64 * hh),
                            skip_group_check=True,
                        )
                    # uT = psum_u + c0 (per-partition bias)
                    nc.vector.tensor_scalar_add(
                        uT[:, r0:r1], psum_u[:, r0:r1], c0_sb
                    )

            # ---- X1T = relu(u[g,j] - v[g,i]) as bf16, pairs = (g, i, j) ----
            x1 = bigp.tile([128, HALF_PAIRS], BF)
            x1p = bigp.tile([128, HALF_PAIRS], BF)
            GSB = GC // 2 // NSBP  # groups per (sbp, half) = 8
            chunks = [(0, 2), (2, GSB // 2), (GSB // 2, GSB)] + [
                (sbp * GSB, (sbp + 1) * GSB) for sbp in range(1, NSBP)
            ]

            def emit_x1_chunk(g0, g1):
                ng = g1 - g0
                gs = slice(g0 * P, g1 * P)
                ps = slice(g0 * P * P, g1 * P * P)
                u3 = uT[:, gs].rearrange("p (g t) -> p g t", t=P)  # t = j
                u4 = u3.unsqueeze(2).broadcast_to([128, ng, P, P])
                v3 = vT[:, gs].rearrange("p (g t) -> p g t", t=P)  # t = i
                v4 = v3.unsqueeze(3).broadcast_to([128, ng, P, P])
                x1p4 = x1p[:, ps].rearrange("p (g i j) -> p g i j", i=P, j=P)
                nc.vector.tensor_tensor(x1p4, u4, v4, op=mybir.AluOpType.subtract)
                nc.vector.tensor_scalar_max(x1[:, ps], x1p[:, ps], 0.0)

            # first three chunks (= sbp0's pairs) up front; later chunks are
            # interleaved into the main loop so DVE work stays spread out
            for g0, g1 in chunks[:3]:
                emit_x1_chunk(g0, g1)

            # ---- main loop ----
            # output is written TRANSPOSED (out_d[bot, row], bf16); the host
            # untransposes during unshard.
            outT = out_d.rearrange("(c p) r -> p c r", p=128)
            with tc.tile_pool(name="psz", bufs=4, space="PSUM") as psz:
                y_hist = {}  # sbp -> [y_t per hh]
                pool_hist = {}  # sbp -> pooledT tile ([128, 2048], hh-split)

                def emit_mms(sbp, c, k):
                    """Fill the two hh tiles of (c, k); K=128 matmuls against
                    the zero-padded weight halves."""
                    ts = [
                        psz.tile([128, 1024], FP, tag="z", name="zt")
                        for _ in range(2)
                    ]
                    for n in range(2):
                        pbase = sbp * SB_PAIRS + k * 1024 + n * 512
                        for hh in range(2):
                            wbase = hh * BOT + c * 128
                            nc.tensor.matmul(
                                ts[hh][:, n * 512 : (n + 1) * 512],
                                W2_sb[:, wbase : wbase + 128],
                                x1[:, pbase : pbase + 512],
                                start=True,
                                stop=True,
                            )
                    return ts

                def emit_act(sbp, c):
                    y_t = y_hist[sbp]
                    bi = BLOCK_BI[c]
                    for k in range(2):
                        ts = emit_mms(sbp, c, k)
                        for hh in range(2):
                            b = bi * 2 + k
                            nc.scalar.activation(
                                y_t[hh][:, b * 1024 : (b + 1) * 1024],
                                ts[hh],
                                mybir.ActivationFunctionType.Relu,
                                bias=b2_sb[:, c : c + 1],
                                scale=1.0,
                            )

                def emit_dve(sbp, c, k):
                    pooledT = pool_hist[sbp]
                    ts = emit_mms(sbp, c, k)
                    for hh in range(2):
                        psl = pooledT[
                            :,
                            hh * 1024 + c * 128 + k * 64 :
                            hh * 1024 + c * 128 + (k + 1) * 64,
                        ]
                        nc.vector.reduce_max(
                            psl,
                            ts[hh].rearrange("p (u j) -> p u j", j=P),
                            axis=mybir.AxisListType.X,
                        )

                def emit_bias(sbp, c, on_act=False):
                    # relu(x + b2) in place; on_act routes it to the scalar
                    # engine (used where ACT has a known sem-wait bubble)
                    pooledT = pool_hist[sbp]
                    for hh in range(2):
                        csl = pooledT[
                            :, hh * 1024 + c * 128 : hh * 1024 + (c + 1) * 128
                        ]
                        if on_act:
                            nc.scalar.activation(
                                csl, csl,
                                mybir.ActivationFunctionType.Relu,
                                bias=b2_sb[:, c : c + 1],
                                scale=1.0,
                            )
                        else:
                            nc.vector.tensor_scalar(
                                csl, csl, b2_sb[:, c : c + 1], 0.0,
                                op0=mybir.AluOpType.add,
                                op1=mybir.AluOpType.max,
                            )

                def emit_tree(sbp, bt, hh):
                    batches = (
                        TREE_BATCHES_LAST if sbp == NSBP - 1 else TREE_BATCHES
                    )
                    cs = batches[bt]
                    y_t = y_hist[sbp]
                    pooledT = pool_hist[sbp]
                    nb = len(cs) * 2
                    m = nb * 64
                    yb0 = BLOCK_BI[cs[0]] * 2 * 1024
                    Y = y_t[hh][:, yb0 : yb0 + nb * 1024].rearrange(
                        "p (m j) -> p m j", j=16
                    )
                    t1 = treep.tile([128, m * 8], BF, tag="t1", name="t1")
                    T1 = t1.rearrange("p (m j) -> p m j", j=8)
                    nc.vector.tensor_tensor(
                        T1, Y[:, :, 0:8], Y[:, :, 8:16], op=mybir.AluOpType.max
                    )
                    t2 = treep.tile([128, m * 4], BF, tag="t2", name="t2")
                    T2 = t2.rearrange("p (m j) -> p m j", j=4)
                    nc.vector.tensor_tensor(
                        T2, T1[:, :, 0:4], T1[:, :, 4:8], op=mybir.AluOpType.max
                    )
                    t3 = treep.tile([128, m * 2], BF, tag="t3", name="t3")
                    T3 = t3.rearrange("p (m j) -> p m j", j=2)
                    nc.vector.tensor_tensor(
                        T3, T2[:, :, 0:2], T2[:, :, 2:4], op=mybir.AluOpType.max
                    )
                    p0 = hh * 1024 + cs[0] * 128
                    nc.vector.tensor_tensor(
                        pooledT[:, p0 : p0 + m], T3[:, :, 0], T3[:, :, 1],
                        op=mybir.AluOpType.max,
                    )

                def emit_out_dma(sbp, hh, c0, c1):
                    """Ship bot-chunks [c0, c1) of half hh."""
                    pooledT = pool_hist[sbp]
                    rowbase = hh * HALF_ROWS + sbp * 128
                    nc.sync.dma_start(
                        outT[:, c0:c1, rowbase : rowbase + 128],
                        pooledT[
                            :, hh * 1024 + c0 * 128 : hh * 1024 + c1 * 128
                        ].rearrange("p (c u) -> p c u", c=c1 - c0),
                    )

                for sbp in range(NSBP):
                    y_hist[sbp] = [
                        yp.tile([128, NB * 1024], BF, tag="y", name="y_t")
                        for _ in range(2)
                    ]
                    pool_hist[sbp] = outp.tile(
                        [128, 2048], BF, tag="pooledT", name="pooledT"
                    )
                    # Emission order doubles as the PSUM ring order and the
                    # per-engine queue order; DVE reduces are placed so they
                    # never sit behind more than ~2us of queued tree work,
                    # keeping the 4-deep ring from stalling ACT.
                    emit_act(sbp, 0)
                    emit_dve(sbp, 3, 0)
                    emit_act(sbp, 1)
                    if sbp > 0:
                        # prev sbp's last tree slots between the c3 reduces
                        emit_tree(sbp - 1, 1, 1)
                        emit_out_dma(sbp - 1, 1, 4, 7)
                    emit_dve(sbp, 3, 1)
                    emit_act(sbp, 2)
                    emit_tree(sbp, 0, 0)
                    if sbp + 3 < len(chunks):
                        # fills DVE idle between T0h0 and the D7 reduces
                        g0, g1 = chunks[sbp + 3]
                        gm = (g0 + g1) // 2
                        emit_x1_chunk(g0, gm)
                        emit_x1_chunk(gm, g1)
                    emit_act(sbp, 4)
                    emit_act(sbp, 5)
                    emit_dve(sbp, 7, 0)
                    emit_dve(sbp, 7, 1)
                    emit_tree(sbp, 0, 1)
                    emit_bias(sbp, 3, on_act=True)
                    emit_bias(sbp, 7)
                    emit_out_dma(sbp, 0, 7, 8)
                    emit_out_dma(sbp, 1, 7, 8)
                    # first halves (c0-c3) of both hh are final now; ship them
                    emit_out_dma(sbp, 0, 0, 4)
                    emit_out_dma(sbp, 1, 0, 4)
                    if sbp < NSBP - 1:
                        emit_act(sbp, 6)
                        emit_tree(sbp, 1, 0)
                        emit_out_dma(sbp, 0, 4, 7)
                    else:
                        # last sbp: the (4,5) trees run before act6's
                        # drains land, leaving only the small (6,) ladders
                        # (+DMA) as the exposed DVE tail.
                        emit_tree(sbp, 1, 0)
                        emit_act(sbp, 6)
                        emit_tree(sbp, 1, 1)
                        emit_tree(sbp, 2, 0)
                        emit_out_dma(sbp, 0, 4, 7)
                        emit_tree(sbp, 2, 1)
                        emit_out_dma(sbp, 1, 4, 7)
    nc.finalize()
    return nc


def _get_nc():
    if "nc" not in _CACHE:
        _CACHE["nc"] = build_nc()
    return _CACHE["nc"]


def kernel(
    in_xy, in_dxdy, h_states, seq_start_end, W_emb, b_emb, W1, b1, W2, b2
):
    pos = np.asarray(in_xy, dtype=np.float32)[-1]  # (B, 2)
    hs = np.asarray(h_states, dtype=np.float32).reshape(B, H)
    W_emb = np.asarray(W_emb, dtype=np.float32)
    b_emb = np.asarray(b_emb, dtype=np.float32)
    W1 = np.asarray(W1, dtype=np.float32)
    b1 = np.asarray(b1, dtype=np.float32)
    W2 = np.asarray(W2, dtype=np.float32)
    b2 = np.asarray(b2, dtype=np.float32)

    A = np.ascontiguousarray(W_emb @ W1[:E])  # (2, H)
    W1b = np.ascontiguousarray(W1[E:])  # (H, H)
    c0 = b_emb @ W1[:E] + b1  # (H,)
    c0d = np.ascontiguousarray(np.concatenate([c0, c0])[:, None])  # (128,1)
    W2z = np.zeros((128, 2 * BOT), np.float32)
    W2z[0:64, 0:BOT] = W2
    W2z[64:128, BOT : 2 * BOT] = W2
    W2d = np.ascontiguousarray(W2z.astype(ml_dtypes.bfloat16))  # (128, 2*BOT)
    b2s = np.ascontiguousarray(b2.reshape(BCH, 128).T)  # (128, BCH)

    in_maps = []
    for cid in range(NCORES):
        rs = slice(cid * RC, (cid + 1) * RC)
        in_maps.append(
            {
                "posT": np.ascontiguousarray(pos[rs].T).astype(ml_dtypes.bfloat16),
                "hT": np.ascontiguousarray(hs[rs].T).astype(ml_dtypes.bfloat16),
                "Amat": A.astype(ml_dtypes.bfloat16),
                "W1b": W1b.astype(ml_dtypes.bfloat16),
                "c0d": c0d,
                "W2d": W2d,
                "b2s": b2s,
            }
        )

    _CACHE["in_maps"] = in_maps
    nc = _get_nc()
    res = run_bass_kernel_spmd(nc, in_maps, core_ids=list(range(NCORES)))
    return np.concatenate(
        [np.asarray(r["out"], dtype=np.float32).T for r in res.results], axis=0
    )


if __name__ == "__main__":
    rng = np.random.default_rng(0)
    inputs = {
        "in_xy": rng.standard_normal((8, B, 2), dtype=np.float32),
        "in_dxdy": rng.standard_normal((8, B, 2), dtype=np.float32),
        "h_states": rng.standard_normal((1, B, H), dtype=np.float32),
        "seq_start_end": np.stack(
            [np.arange(G) * P, np.arange(G) * P + P], axis=1
        ).astype(np.int64),
        "W_emb": rng.standard_normal((2, E), dtype=np.float32),
        "b_emb": np.zeros(E, dtype=np.float32),
        "W1": rng.standard_normal((E + H, H), dtype=np.float32),
        "b1": np.zeros(H, dtype=np.float32),
        "W2": rng.standard_normal((H, BOT), dtype=np.float32),
        "b2": np.zeros(BOT, dtype=np.float32),
    }
    out = kernel(**inputs)
    print(out.shape, out.dtype)


# revision 53
# speedup vs baseline: 1.0304x; 1.0129x over previous
"""Trainium2 Bass kernel for nn_PoolHiddenNet (gnn_message_passing).

Reference computation (uniform contiguous groups of P=16):
    pos = in_xy[-1]                       # (B, 2)
    rel[g,i,j] = pos[g,j] - pos[g,i]
    emb = rel @ W_emb + b_emb             # (G,P,P,E)
    x   = concat([emb, h[g,j]], -1)
    x1  = relu(x @ W1 + b1)               # (G,P,P,H)
    x2  = relu(x1 @ W2 + b2)              # (G,P,P,BOT)
    out = max over j -> (B, BOT)

Algebraic restructuring used here:
    x1[g,i,j] = relu(u[g,j] - v[g,i])
       u[g,r]  = pos[g,r] @ (W_emb @ W1[:E]) + h[g,r] @ W1[E:] + (b_emb @ W1[:E] + b1)
       v[g,r]  = pos[g,r] @ (W_emb @ W1[:E])
    out[g,i]  = max_j relu(x1[g,i,j] @ W2 + b2)      (relu commutes with max)

Sharding: data-parallel over groups; 64 groups (1024 rows) per core.
Device layout: "dup-halves" -- SBUF partitions 0:64 carry the h-dim for the
first 32 groups' data, partitions 64:128 carry the h-dim for the last 32
groups, so all DVE/ACT ops use the full 128 lanes.

The W2 matmuls contract K=128 against zero-padded weights (W2z cols
0:BOT = [W2;0] selects the half-0 x1 rows, cols BOT:2BOT = [0;W2] the
half-1 rows).  Measured on hw, K<=64 matmuls run the PE at half column
rate (427ns vs 216ns per 512 output columns), so padding the contraction
to 128 rows doubles PE throughput for free and drops the tensor engine
from ~103us busy (the previous 3-way bottleneck) to ~75us, leaving ACT
and DVE as the only walls.

Drain schedule (the true bottleneck): every z element must cross
PSUM->SBUF through ACT (1 elem/cycle @1.2GHz) or DVE (1 elem/cycle
@0.96GHz); nothing else on TRN2 can read PSUM (dual-PSUM tensor_tensor
operands are rejected by the bir verifier, DMA cannot read PSUM, and
GpSimd tensor ops fail codegen in this toolchain).  Each PSUM tile
[128, 1024] fp32 is one (bot-chunk c, k-half, hh) = (64 gi x 16 j); a
4-deep PSUM ring keeps the PE ahead of the drains.  Routes:
  ACT (c in {0,1,2,4,5,6}): relu+bias ACTIVATE drains the tile to SBUF
      bf16 y blocks; the 16->1 max tree runs on DVE as bf16 2x
      tensor_tensor ops batched 6 blocks at a time, L4 writing straight
      into pooled.
  DVE (c in {3,7}): reduce_max straight from PSUM (raw), fused
      (add bias, max 0) tensor_scalar afterwards.
The emission order interleaves DVE-routed tiles between ACT c's and
places each tree burst where the ring never waits on a queued DVE
reduce; the previous sbp's last tree fills the DVE idle at each sbp
boundary.  GpSimd does nothing; all DMA runs on sync/scalar queues.
Both ACT and DVE end up ~95% busy; this schedule sits at the measured
ACT+DVE drain wall (~26-27us per superblock-pair).
"""

import sys

import numpy as np

try:
    import concourse.bass as bass
except ImportError:  # pragma: no cover
    sys.path.insert(0, "/opt/trn_rl_repo")
    import concourse.bass as bass

from concourse import bacc

import ml_dtypes

import concourse.mybir as mybir
from concourse.bass_utils import run_bass_kernel_spmd
from concourse.tile import TileContext

# Problem constants (hardcoded per spec)
B, G, P, E, H, BOT = 8192, 512, 16, 64, 64, 1024
NCORES = 8
GC = G // NCORES  # 64 groups per core
RC = GC * P  # 1024 batch rows per core
HALF_ROWS = RC // 2  # 512 rows per half
HALF_PAIRS = (GC // 2) * P * P  # 8192 pairs per half
NSBP = 4  # superblock-pairs; each sbp makes one 128-row block per half
SB_PAIRS = 2048  # pairs per (sbp, half) = 128 rows * 16 j
BCH = BOT // 128  # 8 bot chunks of 128 channels

FP = mybir.dt.float32
BF = mybir.dt.bfloat16

# bot-chunks drained by DVE reduce_max; the rest go to ACT.
DVE_CS = (3, 7)
# ACT c's grouped into tree batches; each batch is a contiguous run of
# pooled columns (pooled col = c*128 + k*64).  The last sbp splits the
# second batch so the post-act6 DVE tail is one small burst instead of
# the full (4,5,6) ladder.
TREE_BATCHES = ((0, 1, 2), (4, 5, 6))
TREE_BATCHES_LAST = ((0, 1, 2), (4, 5), (6,))
# fixed y-block index per ACT c (independent of tree batching)
BLOCK_BI = {0: 0, 1: 1, 2: 2, 4: 3, 5: 4, 6: 5}
NB = 12  # y blocks of 1024 per (sbp, hh)

_CACHE = {}


def build_nc():
    nc = bacc.Bacc("TRN2", target_bir_lowering=False, debug=False, num_devices=NCORES)
    posT_d = nc.declare_dram_parameter("posT", [2, RC], BF, isOutput=False)
    hT_d = nc.declare_dram_parameter("hT", [H, RC], BF, isOutput=False)
    A_d = nc.declare_dram_parameter("Amat", [2, H], BF, isOutput=False)
    W1b_d = nc.declare_dram_parameter("W1b", [H, H], BF, isOutput=False)
    c0_d = nc.declare_dram_parameter("c0d", [128, 1], FP, isOutput=False)
    W2_d = nc.declare_dram_parameter("W2d", [128, 2 * BOT], BF, isOutput=False)
    b2_d = nc.declare_dram_parameter("b2s", [128, BCH], FP, isOutput=False)
    out_d = nc.declare_dram_parameter("out", [BOT, RC], BF, isOutput=True)

    with TileContext(nc) as tc:
        with (
            tc.tile_pool(name="const", bufs=1) as constp,
            tc.tile_pool(name="big", bufs=1) as bigp,
            tc.tile_pool(name="y", bufs=4) as yp,
            tc.tile_pool(name="tree", bufs=2) as treep,
            tc.tile_pool(name="outp", bufs=3) as outp,
        ):
            # ---- constants / inputs to SBUF (split across two DMA queues;
            # gpsimd stays idle so its DGE drain is cheap) ----
            posT = constp.tile([2, RC], BF)
            nc.scalar.dma_start(posT, posT_d[:, :])
            # hT in two pieces: the 32KB strided slab covering prep's first
            # row-slice (cols 0:128 and 512:640 = both halves' rows 0:128)
            # lands ~2.5us before the rest of the 128KB, unblocking the
            # W1b matmul that gates the whole x1/drain chain.
            hT = constp.tile([H, RC], BF)
            hT3 = hT.rearrange("h (b c) -> h b c", c=HALF_ROWS)
            hTd3 = hT_d.rearrange("h (b c) -> h b c", c=HALF_ROWS)
            nc.scalar.dma_start(hT3[:, :, 0:128], hTd3[:, :, 0:128])
            nc.scalar.dma_start(hT3[:, :, 128:HALF_ROWS], hTd3[:, :, 128:HALF_ROWS])
            A_sb = constp.tile([2, H], BF)
            nc.sync.dma_start(A_sb, A_d[:, :])
            W1b_sb = constp.tile([H, H], BF)
            nc.sync.dma_start(W1b_sb, W1b_d[:, :])
            c0_sb = constp.tile([128, 1], FP)
            nc.sync.dma_start(c0_sb, c0_d[:, :])
            b2_sb = constp.tile([128, BCH], FP)
            nc.sync.dma_start(b2_sb, b2_d[:, :])
            W2_sb = constp.tile([128, 2 * BOT], BF)
            nc.sync.dma_start(W2_sb, W2_d[:, :])

            # ---- u/v prep (dup-halves layout) ----
            # uT[p, r'] : h = p % 64 ; r = (p // 64) * 512 + r'
            with tc.tile_pool(name="prepps", bufs=1, space="PSUM") as prepps:
                psum_u = prepps.tile([128, HALF_ROWS], FP)
                vT = constp.tile([128, HALF_ROWS], FP)
                uT = constp.tile([128, HALF_ROWS], FP)
                # v = pos @ A for both halves, one copy out (ACT), then the
                # h-term accumulates on top; uadd runs on DVE.  Done in two
                # row-slices so sbp0's rows (0:128) finish first and the x1
                # build can start early.
                for r0, r1 in ((0, 128), (128, HALF_ROWS)):
                    for hh in range(2):
                        usl = psum_u[64 * hh : 64 * (hh + 1), r0:r1]
                        rs = slice(hh * HALF_ROWS + r0, hh * HALF_ROWS + r1)
                        nc.tensor.matmul(
                            usl, A_sb, posT[:, rs],
                            start=True, stop=True, tile_position=(0, 64 * hh),
                        )
                    nc.scalar.copy(vT[:, r0:r1], psum_u[:, r0:r1])
                    for hh in range(2):
                        usl = psum_u[64 * hh : 64 * (hh + 1), r0:r1]
                        rs = slice(hh * HALF_ROWS + r0, hh * HALF_ROWS + r1)
                        nc.tensor.matmul(
                            usl, W1b_sb, hT[:, rs],
                            start=False, stop=True, tile_position=(0, 64 * hh),
                            skip_group_check=True,
                        )
                    # uT = psum_u + c0 (per-partition bias)
                    nc.vector.tensor_scalar_add(
                        uT[:, r0:r1], psum_u[:, r0:r1], c0_sb
                    )

            # ---- X1T = relu(u[g,j] - v[g,i]) as bf16, pairs = (g, i, j) ----
            x1 = bigp.tile([128, HALF_PAIRS], BF)
            x1p = bigp.tile([128, HALF_PAIRS], BF)
            GSB = GC // 2 // NSBP  # groups per (sbp, half) = 8
            chunks = [(0, 2), (2, GSB // 2), (GSB // 2, GSB)] + [
                (sbp * GSB, (sbp + 1) * GSB) for sbp in range(1, NSBP)
            ]

            def emit_x1_chunk(g0, g1):
                ng = g1 - g0
                gs = slice(g0 * P, g1 * P)
                ps = slice(g0 * P * P, g1 * P * P)
                u3 = uT[:, gs].rearrange("p (g t) -> p g t", t=P)  # t = j
                u4 = u3.unsqueeze(2).broadcast_to([128, ng, P, P])
                v3 = vT[:, gs].rearrange("p (g t) -> p g t", t=P)  # t = i
                v4 = v3.unsqueeze(3).broadcast_to([128, ng, P, P])
                x1p4 = x1p[:, ps].rearrange("p (g i j) -> p g i j", i=P, j=P)
                nc.vector.tensor_tensor(x1p4, u4, v4, op=mybir.AluOpType.subtract)
                nc.vector.tensor_scalar_max(x1[:, ps], x1p[:, ps], 0.0)

            # first three chunks (= sbp0's pairs) up front; later chunks are
            # interleaved into the main loop so DVE work stays spread out
            for g0, g1 in chunks[:3]:
                emit_x1_chunk(g0, g1)

            # ---- main loop ----
            # output is written TRANSPOSED (out_d[bot, row], bf16); the host
            # untransposes during unshard.
            outT = out_d.rearrange("(c p) r -> p c r", p=128)
            with tc.tile_pool(name="psz", bufs=4, space="PSUM") as psz:
                y_hist = {}  # sbp -> [y_t per hh]
                pool_hist = {}  # sbp -> pooledT tile ([128, 2048], hh-split)

                def emit_mms(sbp, c, k):
                    """Fill the two hh tiles of (c, k); K=128 matmuls against
                    the zero-padded weight halves."""
                    ts = [
                        psz.tile([128, 1024], FP, tag="z", name="zt")
                        for _ in range(2)
                    ]
                    for n in range(2):
                        pbase = sbp * SB_PAIRS + k * 1024 + n * 512
                        for hh in range(2):
                            wbase = hh * BOT + c * 128
                            nc.tensor.matmul(
                                ts[hh][:, n * 512 : (n + 1) * 512],
                                W2_sb[:, wbase : wbase + 128],
                                x1[:, pbase : pbase + 512],
                                start=True,
                                stop=True,
                            )
                    return ts

                def emit_act(sbp, c):
                    y_t = y_hist[sbp]
                    bi = BLOCK_BI[c]
                    for k in range(2):
                        ts = emit_mms(sbp, c, k)
                        for hh in range(2):
                            b = bi * 2 + k
                            nc.scalar.activation(
                                y_t[hh][:, b * 1024 : (b + 1) * 1024],
                                ts[hh],
                                mybir.ActivationFunctionType.Relu,
                                bias=b2_sb[:, c : c + 1],
                                scale=1.0,
                            )

                def emit_dve(sbp, c, k):
                    pooledT = pool_hist[sbp]
                    ts = emit_mms(sbp, c, k)
                    for hh in range(2):
                        psl = pooledT[
                            :,
                            hh * 1024 + c * 128 + k * 64 :
                            hh * 1024 + c * 128 + (k + 1) * 64,
                        ]
                        nc.vector.reduce_max(
                            psl,
                            ts[hh].rearrange("p (u j) -> p u j", j=P),
                            axis=mybir.AxisListType.X,
                        )

                def emit_bias(sbp, c, on_act=False):
                    # relu(x + b2) in place; on_act routes it to the scalar
                    # engine (used where ACT has a known sem-wait bubble)
                    pooledT = pool_hist[sbp]
                    for hh in range(2):
                        csl = pooledT[
                            :, hh * 1024 + c * 128 : hh * 1024 + (c + 1) * 128
                        ]
                        if on_act:
                            nc.scalar.activation(
                                csl, csl,
                                mybir.ActivationFunctionType.Relu,
                                bias=b2_sb[:, c : c + 1],
                                scale=1.0,
                            )
                        else:
                            nc.vector.tensor_scalar(
                                csl, csl, b2_sb[:, c : c + 1], 0.0,
                                op0=mybir.AluOpType.add,
                                op1=mybir.AluOpType.max,
                            )

                def emit_tree_L1(sbp, bt, hh):
                    """First tree level only; returns state for the rest.
                    Lets always-ready L1 work slot in front of reduces that
                    are still waiting on fresh matmuls (sbp boundaries)."""
                    batches = (
                        TREE_BATCHES_LAST if sbp == NSBP - 1 else TREE_BATCHES
                    )
                    cs = batches[bt]
                    y_t = y_hist[sbp]
                    nb = len(cs) * 2
                    m = nb * 64
                    yb0 = BLOCK_BI[cs[0]] * 2 * 1024
                    Y = y_t[hh][:, yb0 : yb0 + nb * 1024].rearrange(
                        "p (m j) -> p m j", j=16
                    )
                    t1 = treep.tile([128, m * 8], BF, tag="t1", name="t1")
                    T1 = t1.rearrange("p (m j) -> p m j", j=8)
                    nc.vector.tensor_tensor(
                        T1, Y[:, :, 0:8], Y[:, :, 8:16], op=mybir.AluOpType.max
                    )
                    return (cs, m, t1)

                def emit_tree_rest(sbp, hh, st):
                    cs, m, t1 = st
                    pooledT = pool_hist[sbp]
                    T1 = t1.rearrange("p (m j) -> p m j", j=8)
                    t2 = treep.tile([128, m * 4], BF, tag="t2", name="t2")
                    T2 = t2.rearrange("p (m j) -> p m j", j=4)
                    nc.vector.tensor_tensor(
                        T2, T1[:, :, 0:4], T1[:, :, 4:8], op=mybir.AluOpType.max
                    )
                    t3 = treep.tile([128, m * 2], BF, tag="t3", name="t3")
                    T3 = t3.rearrange("p (m j) -> p m j", j=2)
                    nc.vector.tensor_tensor(
                        T3, T2[:, :, 0:2], T2[:, :, 2:4], op=mybir.AluOpType.max
                    )
                    p0 = hh * 1024 + cs[0] * 128
                    nc.vector.tensor_tensor(
                        pooledT[:, p0 : p0 + m], T3[:, :, 0], T3[:, :, 1],
                        op=mybir.AluOpType.max,
                    )

                def emit_tree(sbp, bt, hh):
                    emit_tree_rest(sbp, hh, emit_tree_L1(sbp, bt, hh))

                def emit_out_dma(sbp, hh, c0, c1):
                    """Ship bot-chunks [c0, c1) of half hh."""
                    pooledT = pool_hist[sbp]
                    rowbase = hh * HALF_ROWS + sbp * 128
                    nc.sync.dma_start(
                        outT[:, c0:c1, rowbase : rowbase + 128],
                        pooledT[
                            :, hh * 1024 + c0 * 128 : hh * 1024 + c1 * 128
                        ].rearrange("p (c u) -> p c u", c=c1 - c0),
                    )

                for sbp in range(NSBP):
                    y_hist[sbp] = [
                        yp.tile([128, NB * 1024], BF, tag="y", name="y_t")
                        for _ in range(2)
                    ]
                    pool_hist[sbp] = outp.tile(
                        [128, 2048], BF, tag="pooledT", name="pooledT"
                    )
                    # Emission order doubles as the PSUM ring order and the
                    # per-engine queue order; DVE reduces are placed so they
                    # never sit behind more than ~2us of queued tree work,
                    # keeping the 4-deep ring from stalling ACT.
                    emit_act(sbp, 0)
                    if sbp > 0:
                        # prev sbp's last tree: L1 (always-ready) runs while
                        # the c3 reduces still wait on their matmuls; the
                        # remaining levels slot between the reduces as before
                        bstate = emit_tree_L1(sbp - 1, 1, 1)
                    emit_dve(sbp, 3, 0)
                    emit_act(sbp, 1)
                    if sbp > 0:
                        emit_tree_rest(sbp - 1, 1, bstate)
                        emit_out_dma(sbp - 1, 1, 4, 7)
                    emit_dve(sbp, 3, 1)
                    emit_act(sbp, 2)
                    emit_tree(sbp, 0, 0)
                    if sbp + 3 < len(chunks):
                        # fills DVE idle between T0h0 and the D7 reduces
                        g0, g1 = chunks[sbp + 3]
                        gm = (g0 + g1) // 2
                        emit_x1_chunk(g0, gm)
                        emit_x1_chunk(gm, g1)
                    emit_act(sbp, 4)
                    emit_act(sbp, 5)
                    emit_dve(sbp, 7, 0)
                    emit_dve(sbp, 7, 1)
                    emit_tree(sbp, 0, 1)
                    emit_bias(sbp, 3, on_act=True)
                    emit_bias(sbp, 7)
                    emit_out_dma(sbp, 0, 7, 8)
                    emit_out_dma(sbp, 1, 7, 8)
                    # first halves (c0-c3) of both hh are final now; ship them
                    emit_out_dma(sbp, 0, 0, 4)
                    emit_out_dma(sbp, 1, 0, 4)
                    if sbp < NSBP - 1:
                        emit_act(sbp, 6)
                        emit_tree(sbp, 1, 0)
                        emit_out_dma(sbp, 0, 4, 7)
                    else:
                        # last sbp: the (4,5) trees run before act6's
                        # drains land, leaving only the small (6,) ladders
                        # (+DMA) as the exposed DVE tail.
                        emit_tree(sbp, 1, 0)
                        emit_act(sbp, 6)
                        emit_tree(sbp, 1, 1)
                        emit_tree(sbp, 2, 0)
                        emit_out_dma(sbp, 0, 4, 7)
                        emit_tree(sbp, 2, 1)
                        emit_out_dma(sbp, 1, 4, 7)
    nc.finalize()
    return nc


def _get_nc():
    if "nc" not in _CACHE:
        _CACHE["nc"] = build_nc()
    return _CACHE["nc"]


def kernel(
    in_xy, in_dxdy, h_states, seq_start_end, W_emb, b_emb, W1, b1, W2, b2
):
    pos = np.asarray(in_xy, dtype=np.float32)[-1]  # (B, 2)
    hs = np.asarray(h_states, dtype=np.float32).reshape(B, H)
    W_emb = np.asarray(W_emb, dtype=np.float32)
    b_emb = np.asarray(b_emb, dtype=np.float32)
    W1 = np.asarray(W1, dtype=np.float32)
    b1 = np.asarray(b1, dtype=np.float32)
    W2 = np.asarray(W2, dtype=np.float32)
    b2 = np.asarray(b2, dtype=np.float32)

    A = np.ascontiguousarray(W_emb @ W1[:E])  # (2, H)
    W1b = np.ascontiguousarray(W1[E:])  # (H, H)
    c0 = b_emb @ W1[:E] + b1  # (H,)
    c0d = np.ascontiguousarray(np.concatenate([c0, c0])[:, None])  # (128,1)
    W2z = np.zeros((128, 2 * BOT), np.float32)
    W2z[0:64, 0:BOT] = W2
    W2z[64:128, BOT : 2 * BOT] = W2
    W2d = np.ascontiguousarray(W2z.astype(ml_dtypes.bfloat16))  # (128, 2*BOT)
    b2s = np.ascontiguousarray(b2.reshape(BCH, 128).T)  # (128, BCH)

    in_maps = []
    for cid in range(NCORES):
        rs = slice(cid * RC, (cid + 1) * RC)
        in_maps.append(
            {
                "posT": np.ascontiguousarray(pos[rs].T).astype(ml_dtypes.bfloat16),
                "hT": np.ascontiguousarray(hs[rs].T).astype(ml_dtypes.bfloat16),
                "Amat": A.astype(ml_dtypes.bfloat16),
                "W1b": W1b.astype(ml_dtypes.bfloat16),
                "c0d": c0d,
                "W2d": W2d,
                "b2s": b2s,
            }
        )

    _CACHE["in_maps"] = in_maps
    nc = _get_nc()
    res = run_bass_kernel_spmd(nc, in_maps, core_ids=list(range(NCORES)))
    return np.concatenate(
        [np.asarray(r["out"], dtype=np.float32).T for r in res.results], axis=0
    )


if __name__ == "__main__":
    rng = np.random.default_rng(0)
    inputs = {
        "in_xy": rng.standard_normal((8, B, 2), dtype=np.float32),
        "in_dxdy": rng.standard_normal((8, B, 2), dtype=np.float32),
        "h_states": rng.standard_normal((1, B, H), dtype=np.float32),
        "seq_start_end": np.stack(
            [np.arange(G) * P, np.arange(G) * P + P], axis=1
        ).astype(np.int64),
        "W_emb": rng.standard_normal((2, E), dtype=np.float32),
        "b_emb": np.zeros(E, dtype=np.float32),
        "W1": rng.standard_normal((E + H, H), dtype=np.float32),
        "b1": np.zeros(H, dtype=np.float32),
        "W2": rng.standard_normal((H, BOT), dtype=np.float32),
        "b2": np.zeros(BOT, dtype=np.float32),
    }
    out = kernel(**inputs)
    print(out.shape, out.dtype)
